# revision 15
# baseline (speedup 1.0000x reference)
"""Episodic-memory retrieval kernel for 8 Trainium2 NeuronCores.

Pipeline (classic sharded ANN retrieval, capacity-axis sharding):
  K1: per-core cosine sim (queries x embedding-shard) + local top-8
  host: merge candidates -> global top-8, build gather/scatter indices
  K2: per-core indirect-DMA gather of owned winners from episode shards,
      scatter into a compacted per-core output; host reassembles.
"""

import numpy as np

import concourse.bass as bass
import concourse.bacc as bacc
import concourse.mybir as mybir
from concourse.bass import IndirectOffsetOnAxis
from concourse.bass_utils import run_bass_kernel_spmd
from concourse.masks import make_identity
from concourse.tile import TileContext

F32 = mybir.dt.float32
I32 = mybir.dt.int32
U32 = mybir.dt.uint32

B, H, C, S, CS, K = 256, 256, 16384, 64, 32, 8
NCORES = 8
CSH = C // NCORES        # 2048 episodes per core shard
EPS = 1e-8
NB = 8                   # K2 batches of 128 winner-slots -> capacity 512/core
CAP = NB * 128
ROW = S * H              # 16384 f32 per full episode
NG1 = 4                  # K1 candidate groups (512 episodes each)
CROW = CS * H            # 8192 f32 per compressed episode

_progs = {}


def _build_k1():
    # Outputs per-group top-8 candidates (4 groups of 512 episodes): the
    # host merges across groups and cores. Normalization is folded into the
    # transpose step: transpose-with-diag(1/norm) via a regular matmul.
    nc = bacc.Bacc(None, target_bir_lowering=False)
    q = nc.dram_tensor("q", [B, H], F32, kind="ExternalInput")
    e = nc.dram_tensor("e", [CSH, H], F32, kind="ExternalInput")
    scores = nc.dram_tensor("scores", [B, NG1 * K], F32, kind="ExternalOutput")
    idx = nc.dram_tensor("idx", [B, NG1 * K], U32, kind="ExternalOutput")

    NKT = H // 128   # 2 contraction tiles
    NMT = B // 128   # 2 query tiles

    with TileContext(nc) as tc:
        with (
            tc.tile_pool(name="work", bufs=4) as wp,
            tc.tile_pool(name="psum", bufs=2, space="PSUM") as pp,
            tc.tile_pool(name="persist", bufs=1) as keep,
        ):
            ident = keep.tile([128, 128], F32, tag="ident")
            make_identity(nc, ident[:])

            def rn_of(ssq, n, tag):
                # ssq [128, n] -> 1/max(sqrt(ssq), EPS), batched small ops
                nrm = wp.tile([128, n], F32, tag=f"nrm_{tag}")
                nc.scalar.activation(
                    out=nrm[:], in_=ssq[:],
                    func=mybir.ActivationFunctionType.Sqrt)
                nc.vector.tensor_scalar_max(nrm[:], nrm[:], EPS)
                rn = wp.tile([128, n], F32, tag=f"rn_{tag}")
                nc.vector.reciprocal(rn[:], nrm[:])
                return rn

            # ---- queries: load, ssq, rn, diag-scaled transpose
            qT = [keep.tile([128, B], F32, tag=f"qT{i}", name=f"qT{i}")
                  for i in range(NKT)]
            qts = []
            qssq = wp.tile([128, NMT], F32, tag="qssq")
            for mt in range(NMT):
                qt = wp.tile([128, H], F32, tag=f"qt{mt}", name=f"qt{mt}")
                nc.sync.dma_start(out=qt[:], in_=q[mt * 128:(mt + 1) * 128, :])
                sq = wp.tile([128, H], F32, tag="sq")
                nc.scalar.activation(
                    out=sq[:], in_=qt[:],
                    func=mybir.ActivationFunctionType.Square,
                    accum_out=qssq[:, mt:mt + 1])
                qts.append(qt)
            qrn = rn_of(qssq, NMT, "q")
            for mt in range(NMT):
                dg = wp.tile([128, 128], F32, tag=f"dgq{mt}", name=f"dgq{mt}")
                nc.vector.tensor_scalar_mul(dg[:], ident[:], qrn[:, mt:mt + 1])
                pq = pp.tile([128, B], F32, space="PSUM", tag="pq", bufs=1)
                for kt in range(NKT):
                    nc.tensor.matmul(
                        out=pq[:, kt * 128:(kt + 1) * 128],
                        lhsT=qts[mt][:, kt * 128:(kt + 1) * 128],
                        rhs=dg[:], start=True, stop=True)
                for kt in range(NKT):
                    nc.vector.tensor_copy(
                        out=qT[kt][:, mt * 128:(mt + 1) * 128],
                        in_=pq[:, kt * 128:(kt + 1) * 128])

            # ---- episodes by group of 4 tiles: diag-scaled transposes
            eT = [keep.tile([128, CSH], F32, tag=f"eT{i}", name=f"eT{i}")
                  for i in range(NKT)]
            for g in range(NG1):
                ets = []
                essq = wp.tile([128, 4], F32, tag="essq")
                for t in range(4):
                    ct = g * 4 + t
                    et = wp.tile([128, H], F32, tag=f"et{t}", name=f"et{t}")
                    nc.sync.dma_start(
                        out=et[:], in_=e[ct * 128:(ct + 1) * 128, :])
                    sq = wp.tile([128, H], F32, tag="sq")
                    nc.scalar.activation(
                        out=sq[:], in_=et[:],
                        func=mybir.ActivationFunctionType.Square,
                        accum_out=essq[:, t:t + 1])
                    ets.append(et)
                ern = rn_of(essq, 4, "e")
                pts = [pp.tile([128, 512], F32, space="PSUM", tag="pt",
                               name=f"pt{g}_{kt}") for kt in range(NKT)]
                for t in range(4):
                    dg = wp.tile([128, 128], F32, tag=f"dge{t}", name=f"dge{t}")
                    nc.vector.tensor_scalar_mul(dg[:], ident[:], ern[:, t:t + 1])
                    for kt in range(NKT):
                        nc.tensor.matmul(
                            out=pts[kt][:, t * 128:(t + 1) * 128],
                            lhsT=ets[t][:, kt * 128:(kt + 1) * 128],
                            rhs=dg[:], start=True, stop=True)
                for kt in range(NKT):
                    nc.vector.tensor_copy(
                        out=eT[kt][:, g * 512:(g + 1) * 512], in_=pts[kt][:])

            # ---- sim matmuls + per-group top-8
            sims = [keep.tile([128, CSH], F32, tag=f"sim{mt}", name=f"sim{mt}")
                    for mt in range(NMT)]
            s8s = [keep.tile([128, NG1 * K], F32, tag=f"s8{mt}", name=f"s8{mt}")
                   for mt in range(NMT)]
            i8s = [keep.tile([128, NG1 * K], U32, tag=f"i8{mt}", name=f"i8{mt}")
                   for mt in range(NMT)]
            for g in range(NG1):
                for mt in range(NMT):
                    ps = pp.tile([128, 512], F32, space="PSUM", tag="ps")
                    for kt in range(NKT):
                        nc.tensor.matmul(
                            out=ps[:],
                            lhsT=qT[kt][:, mt * 128:(mt + 1) * 128],
                            rhs=eT[kt][:, g * 512:(g + 1) * 512],
                            start=(kt == 0), stop=(kt == NKT - 1))
                    sl = sims[mt][:, g * 512:(g + 1) * 512]
                    nc.vector.tensor_copy(out=sl, in_=ps[:])
                    nc.vector.max(out=s8s[mt][:, g * K:(g + 1) * K], in_=sl)
                    nc.vector.max_index(
                        out=i8s[mt][:, g * K:(g + 1) * K],
                        in_max=s8s[mt][:, g * K:(g + 1) * K], in_values=sl)

            for mt in range(NMT):
                nc.sync.dma_start(
                    out=scores[mt * 128:(mt + 1) * 128, :], in_=s8s[mt][:])
                nc.sync.dma_start(
                    out=idx[mt * 128:(mt + 1) * 128, :], in_=i8s[mt][:])

    nc.compile()
    return nc


def _build_k2(scatter_queue=1):
    # tbl rows: [0, 2*CSH) = episode half-rows (2i, 2i+1), [2*CSH, 3*CSH) =
    # compressed rows. One gather per winner half, two independent
    # gather->scatter chains per batch. Scatters go on SWDGE queue 1 so the
    # SDMA engines round-robin between gather and scatter rings.
    nc = bacc.Bacc(None, target_bir_lowering=False,
                   num_swdge_queues=2 if scatter_queue else 1)
    tbl = nc.dram_tensor("tbl", [3 * CSH, CROW], F32, kind="ExternalInput")
    # idxs[:, t, b]: t = 0:g0 1:g1 2:slot0 3:slot1
    idxs = nc.dram_tensor("idxs", [128, 4, NB], I32, kind="ExternalInput")
    out = nc.dram_tensor("out", [CAP, ROW], F32, kind="ExternalOutput")
    out_half = out[:].rearrange("w (h x) -> (w h) x", h=2)  # [2*CAP, CROW]

    with TileContext(nc) as tc:
        with (
            tc.tile_pool(name="d0", bufs=3) as p0,
            tc.tile_pool(name="d1", bufs=3) as p1,
            tc.tile_pool(name="idxp", bufs=1) as ip,
        ):
            ix = ip.tile([128, 4, NB], I32, tag="ix")
            nc.sync.dma_start(out=ix[:], in_=idxs[:])

            for bt in range(NB):
                t0 = p0.tile([128, CROW], F32, tag="t0")
                t1 = p1.tile([128, CROW], F32, tag="t1")
                # half0 of every owned winner (episode half or compressed row)
                nc.gpsimd.indirect_dma_start(
                    out=t0[:],
                    out_offset=None,
                    in_=tbl[:],
                    in_offset=IndirectOffsetOnAxis(ap=ix[:, 0, bt:bt + 1], axis=0),
                    bounds_check=3 * CSH - 1,
                    oob_is_err=False,
                )
                # half1 (episode half-row 2i+1) for non-compressed winners
                nc.gpsimd.indirect_dma_start(
                    out=t1[:],
                    out_offset=None,
                    in_=tbl[:],
                    in_offset=IndirectOffsetOnAxis(ap=ix[:, 1, bt:bt + 1], axis=0),
                    bounds_check=3 * CSH - 1,
                    oob_is_err=False,
                )
                sc0 = nc.gpsimd.indirect_dma_start(
                    out=out_half,
                    out_offset=IndirectOffsetOnAxis(ap=ix[:, 2, bt:bt + 1], axis=0),
                    in_=t0[:],
                    in_offset=None,
                    bounds_check=2 * CAP - 1,
                    oob_is_err=False,
                )
                # compressed winners' half1 stays zero (pre-zeroed output)
                sc1 = nc.gpsimd.indirect_dma_start(
                    out=out_half,
                    out_offset=IndirectOffsetOnAxis(ap=ix[:, 3, bt:bt + 1], axis=0),
                    in_=t1[:],
                    in_offset=None,
                    bounds_check=2 * CAP - 1,
                    oob_is_err=False,
                )
                if scatter_queue:
                    sc0.ins.queue = f"qPoolDynamic{scatter_queue}"
                    sc1.ins.queue = f"qPoolDynamic{scatter_queue}"

    nc.compile()
    return nc


def _get(name):
    if name not in _progs:
        _progs[name] = _build_k1() if name == "k1" else _build_k2()
    return _progs[name]


def _run_k1(query, emb, trace=False):
    nc = _get("k1")
    q = np.ascontiguousarray(query, dtype=np.float32)
    in_maps = [
        {"q": q, "e": np.ascontiguousarray(emb[c * CSH:(c + 1) * CSH])}
        for c in range(NCORES)
    ]
    return run_bass_kernel_spmd(
        nc, in_maps, core_ids=list(range(NCORES)), trace=trace
    )


def _run_k2(in_maps, trace=False):
    nc = _get("k2")
    return run_bass_kernel_spmd(
        nc, in_maps, core_ids=list(range(NCORES)), trace=trace
    )


def kernel(query, episode_embeddings, episodes, compressed_episodes,
           is_compressed, k, _trace=False, _results=None):
    assert int(k) == K
    r1 = _run_k1(query, episode_embeddings, trace=_trace)

    # host: merge the 8 per-shard candidate lists -> global top-8
    cand_s = np.concatenate(
        [r1.results[c]["scores"] for c in range(NCORES)], axis=1
    )  # [B, NCORES*NG1*K]
    goff = (np.arange(NG1 * K) // K) * 512   # group offset within a shard
    cand_i = np.concatenate(
        [r1.results[c]["idx"].astype(np.int64) + goff[None, :] + c * CSH
         for c in range(NCORES)],
        axis=1,
    )
    order = np.argsort(-cand_s, axis=1, kind="stable")[:, :K]
    top_scores = np.take_along_axis(cand_s, order, axis=1)
    top_idx = np.take_along_axis(cand_i, order, axis=1)  # [B, K] global

    # host: per-core gather/scatter index tensors
    comp = np.asarray(is_compressed).astype(bool)
    flat_idx = top_idx.reshape(-1)              # [B*K] winner slot w -> episode
    flat_comp = comp[flat_idx]
    owner = flat_idx // CSH
    ep_half = np.asarray(episodes, dtype=np.float32).reshape(C, 2, CROW)
    cp_flat = np.asarray(compressed_episodes, dtype=np.float32).reshape(C, CROW)

    in2 = []
    owned = []
    for c in range(NCORES):
        w = np.nonzero(owner == c)[0]           # winner slots owned by core c
        n = len(w)
        assert n <= CAP, f"core {c} owns {n} winners > capacity {CAP}"
        li = (flat_idx[w] - c * CSH).astype(np.int64)   # local table rows
        fc = flat_comp[w]
        # spread winners across batches, and within each batch interleave
        # the active rows evenly over all 128 partitions (SDMA engine balance)
        j = np.arange(n)
        b = j % NB
        i = j // NB
        mb = np.array([(n - bb + NB - 1) // NB for bb in range(NB)])
        mb = np.maximum(mb, 1)
        p = (i * 128) // mb[b]
        rank = b * 128 + p
        owned.append((w, rank))
        OOBT = np.int32(3 * CSH)
        OOBS = np.int32(2 * CAP)
        g0v = np.full(CAP, OOBT, np.int32)
        g1v = np.full(CAP, OOBT, np.int32)
        s0v = np.full(CAP, OOBS, np.int32)
        s1v = np.full(CAP, OOBS, np.int32)
        g0v[rank] = np.where(fc, 2 * CSH + li, 2 * li)
        g1v[rank] = np.where(fc, OOBT, 2 * li + 1)
        s0v[rank] = 2 * rank
        s1v[rank] = np.where(fc, OOBS, 2 * rank + 1)
        tbl = np.concatenate(
            [ep_half[c * CSH:(c + 1) * CSH].reshape(2 * CSH, CROW),
             cp_flat[c * CSH:(c + 1) * CSH]], axis=0
        )
        # device tensor idxs[p, t, bt]; rank = bt*128 + p
        iv = np.stack([g0v, g1v, s0v, s1v], axis=1).reshape(NB, 128, 4)
        in2.append({
            "tbl": tbl,
            "idxs": np.ascontiguousarray(iv.transpose(1, 2, 0)),
        })

    r2 = _run_k2(in2, trace=_trace)

    retrieved = np.empty((B * K, S, H), dtype=np.float32)
    for c in range(NCORES):
        w, rank = owned[c]
        retrieved[w] = r2.results[c]["out"][rank].reshape(-1, S, H)
    retrieved = retrieved.reshape(B, K, S, H)

    if _results is not None:
        _results["r1"] = r1
        _results["r2"] = r2
    return retrieved, top_scores.astype(np.float32)


# revision 16
# speedup vs baseline: 1.1441x; 1.1441x over previous
"""Episodic-memory retrieval kernel for 8 Trainium2 NeuronCores.

Pipeline (classic sharded ANN retrieval, capacity-axis sharding):
  K1: per-core cosine sim (queries x embedding-shard) + local top-8
  host: merge candidates -> global top-8, build gather/scatter indices
  K2: per-core indirect-DMA gather of owned winners from episode shards,
      scatter into a compacted per-core output; host reassembles.
"""

import numpy as np

import concourse.bass as bass
import concourse.bacc as bacc
import concourse.mybir as mybir
from concourse.bass import IndirectOffsetOnAxis
from concourse.bass_utils import run_bass_kernel_spmd
from concourse.masks import make_identity
from concourse.tile import TileContext

F32 = mybir.dt.float32
I32 = mybir.dt.int32
U32 = mybir.dt.uint32

B, H, C, S, CS, K = 256, 256, 16384, 64, 32, 8
NCORES = 8
CSH = C // NCORES        # 2048 episodes per core shard
EPS = 1e-8
NB = 3                   # K2 batches of 128 winner-slots -> capacity 512/core
CAP = NB * 128
ROW = S * H              # 16384 f32 per full episode
NG1 = 4                  # K1 candidate groups (512 episodes each)
CROW = CS * H            # 8192 f32 per compressed episode

_progs = {}


def _build_k1():
    # Outputs per-group top-8 candidates (4 groups of 512 episodes): the
    # host merges across groups and cores. Normalization is folded into the
    # transpose step: transpose-with-diag(1/norm) via a regular matmul.
    nc = bacc.Bacc(None, target_bir_lowering=False)
    q = nc.dram_tensor("q", [B, H], F32, kind="ExternalInput")
    e = nc.dram_tensor("e", [CSH, H], F32, kind="ExternalInput")
    scores = nc.dram_tensor("scores", [B, NG1 * K], F32, kind="ExternalOutput")
    idx = nc.dram_tensor("idx", [B, NG1 * K], U32, kind="ExternalOutput")

    NKT = H // 128   # 2 contraction tiles
    NMT = B // 128   # 2 query tiles

    with TileContext(nc) as tc:
        with (
            tc.tile_pool(name="work", bufs=4) as wp,
            tc.tile_pool(name="psum", bufs=2, space="PSUM") as pp,
            tc.tile_pool(name="persist", bufs=1) as keep,
        ):
            ident = keep.tile([128, 128], F32, tag="ident")
            make_identity(nc, ident[:])

            def rn_of(ssq, n, tag):
                # ssq [128, n] -> 1/max(sqrt(ssq), EPS), batched small ops
                nrm = wp.tile([128, n], F32, tag=f"nrm_{tag}")
                nc.scalar.activation(
                    out=nrm[:], in_=ssq[:],
                    func=mybir.ActivationFunctionType.Sqrt)
                nc.vector.tensor_scalar_max(nrm[:], nrm[:], EPS)
                rn = wp.tile([128, n], F32, tag=f"rn_{tag}")
                nc.vector.reciprocal(rn[:], nrm[:])
                return rn

            # ---- queries: load, ssq, rn, diag-scaled transpose
            qT = [keep.tile([128, B], F32, tag=f"qT{i}", name=f"qT{i}")
                  for i in range(NKT)]
            qts = []
            qssq = wp.tile([128, NMT], F32, tag="qssq")
            for mt in range(NMT):
                qt = wp.tile([128, H], F32, tag=f"qt{mt}", name=f"qt{mt}")
                nc.sync.dma_start(out=qt[:], in_=q[mt * 128:(mt + 1) * 128, :])
                sq = wp.tile([128, H], F32, tag="sq")
                nc.scalar.activation(
                    out=sq[:], in_=qt[:],
                    func=mybir.ActivationFunctionType.Square,
                    accum_out=qssq[:, mt:mt + 1])
                qts.append(qt)
            qrn = rn_of(qssq, NMT, "q")
            for mt in range(NMT):
                dg = wp.tile([128, 128], F32, tag=f"dgq{mt}", name=f"dgq{mt}")
                nc.vector.tensor_scalar_mul(dg[:], ident[:], qrn[:, mt:mt + 1])
                pq = pp.tile([128, B], F32, space="PSUM", tag="pq", bufs=1)
                for kt in range(NKT):
                    nc.tensor.matmul(
                        out=pq[:, kt * 128:(kt + 1) * 128],
                        lhsT=qts[mt][:, kt * 128:(kt + 1) * 128],
                        rhs=dg[:], start=True, stop=True)
                for kt in range(NKT):
                    nc.vector.tensor_copy(
                        out=qT[kt][:, mt * 128:(mt + 1) * 128],
                        in_=pq[:, kt * 128:(kt + 1) * 128])

            # ---- episodes by group of 4 tiles: diag-scaled transposes
            eT = [keep.tile([128, CSH], F32, tag=f"eT{i}", name=f"eT{i}")
                  for i in range(NKT)]
            for g in range(NG1):
                ets = []
                essq = wp.tile([128, 4], F32, tag="essq")
                for t in range(4):
                    ct = g * 4 + t
                    et = wp.tile([128, H], F32, tag=f"et{t}", name=f"et{t}")
                    nc.sync.dma_start(
                        out=et[:], in_=e[ct * 128:(ct + 1) * 128, :])
                    sq = wp.tile([128, H], F32, tag="sq")
                    nc.scalar.activation(
                        out=sq[:], in_=et[:],
                        func=mybir.ActivationFunctionType.Square,
                        accum_out=essq[:, t:t + 1])
                    ets.append(et)
                ern = rn_of(essq, 4, "e")
                pts = [pp.tile([128, 512], F32, space="PSUM", tag="pt",
                               name=f"pt{g}_{kt}") for kt in range(NKT)]
                for t in range(4):
                    dg = wp.tile([128, 128], F32, tag=f"dge{t}", name=f"dge{t}")
                    nc.vector.tensor_scalar_mul(dg[:], ident[:], ern[:, t:t + 1])
                    for kt in range(NKT):
                        nc.tensor.matmul(
                            out=pts[kt][:, t * 128:(t + 1) * 128],
                            lhsT=ets[t][:, kt * 128:(kt + 1) * 128],
                            rhs=dg[:], start=True, stop=True)
                for kt in range(NKT):
                    nc.vector.tensor_copy(
                        out=eT[kt][:, g * 512:(g + 1) * 512], in_=pts[kt][:])

            # ---- sim matmuls + per-group top-8
            sims = [keep.tile([128, CSH], F32, tag=f"sim{mt}", name=f"sim{mt}")
                    for mt in range(NMT)]
            s8s = [keep.tile([128, NG1 * K], F32, tag=f"s8{mt}", name=f"s8{mt}")
                   for mt in range(NMT)]
            i8s = [keep.tile([128, NG1 * K], U32, tag=f"i8{mt}", name=f"i8{mt}")
                   for mt in range(NMT)]
            for g in range(NG1):
                for mt in range(NMT):
                    ps = pp.tile([128, 512], F32, space="PSUM", tag="ps")
                    for kt in range(NKT):
                        nc.tensor.matmul(
                            out=ps[:],
                            lhsT=qT[kt][:, mt * 128:(mt + 1) * 128],
                            rhs=eT[kt][:, g * 512:(g + 1) * 512],
                            start=(kt == 0), stop=(kt == NKT - 1))
                    sl = sims[mt][:, g * 512:(g + 1) * 512]
                    nc.vector.tensor_copy(out=sl, in_=ps[:])
                    nc.vector.max(out=s8s[mt][:, g * K:(g + 1) * K], in_=sl)
                    nc.vector.max_index(
                        out=i8s[mt][:, g * K:(g + 1) * K],
                        in_max=s8s[mt][:, g * K:(g + 1) * K], in_values=sl)

            for mt in range(NMT):
                nc.sync.dma_start(
                    out=scores[mt * 128:(mt + 1) * 128, :], in_=s8s[mt][:])
                nc.sync.dma_start(
                    out=idx[mt * 128:(mt + 1) * 128, :], in_=i8s[mt][:])

    nc.compile()
    return nc


def _build_k2(scatter_queue=1):
    # tbl rows: [0, 2*CSH) = episode half-rows (2i, 2i+1), [2*CSH, 3*CSH) =
    # compressed rows. One gather per winner half, two independent
    # gather->scatter chains per batch. Scatters go on SWDGE queue 1 so the
    # SDMA engines round-robin between gather and scatter rings.
    nc = bacc.Bacc(None, target_bir_lowering=False,
                   num_swdge_queues=2 if scatter_queue else 1)
    tbl = nc.dram_tensor("tbl", [3 * CSH, CROW], F32, kind="ExternalInput")
    # idxs[:, t, b]: t = 0:g0 1:g1 2:slot0 3:slot1
    idxs = nc.dram_tensor("idxs", [128, 4, NB], I32, kind="ExternalInput")
    out = nc.dram_tensor("out", [CAP, ROW], F32, kind="ExternalOutput")
    out_half = out[:].rearrange("w (h x) -> (w h) x", h=2)  # [2*CAP, CROW]

    with TileContext(nc) as tc:
        with (
            tc.tile_pool(name="d0", bufs=3) as p0,
            tc.tile_pool(name="d1", bufs=3) as p1,
            tc.tile_pool(name="idxp", bufs=1) as ip,
        ):
            ix = ip.tile([128, 4, NB], I32, tag="ix")
            nc.sync.dma_start(out=ix[:], in_=idxs[:])

            for bt in range(NB):
                t0 = p0.tile([128, CROW], F32, tag="t0")
                t1 = p1.tile([128, CROW], F32, tag="t1")
                # half0 of every owned winner (episode half or compressed row)
                nc.gpsimd.indirect_dma_start(
                    out=t0[:],
                    out_offset=None,
                    in_=tbl[:],
                    in_offset=IndirectOffsetOnAxis(ap=ix[:, 0, bt:bt + 1], axis=0),
                    bounds_check=3 * CSH - 1,
                    oob_is_err=False,
                )
                # half1 (episode half-row 2i+1) for non-compressed winners
                nc.gpsimd.indirect_dma_start(
                    out=t1[:],
                    out_offset=None,
                    in_=tbl[:],
                    in_offset=IndirectOffsetOnAxis(ap=ix[:, 1, bt:bt + 1], axis=0),
                    bounds_check=3 * CSH - 1,
                    oob_is_err=False,
                )
                sc0 = nc.gpsimd.indirect_dma_start(
                    out=out_half,
                    out_offset=IndirectOffsetOnAxis(ap=ix[:, 2, bt:bt + 1], axis=0),
                    in_=t0[:],
                    in_offset=None,
                    bounds_check=2 * CAP - 1,
                    oob_is_err=False,
                )
                # compressed winners' half1 stays zero (pre-zeroed output)
                sc1 = nc.gpsimd.indirect_dma_start(
                    out=out_half,
                    out_offset=IndirectOffsetOnAxis(ap=ix[:, 3, bt:bt + 1], axis=0),
                    in_=t1[:],
                    in_offset=None,
                    bounds_check=2 * CAP - 1,
                    oob_is_err=False,
                )
                if scatter_queue:
                    sc0.ins.queue = f"qPoolDynamic{scatter_queue}"
                    sc1.ins.queue = f"qPoolDynamic{scatter_queue}"

    nc.compile()
    return nc


def _get(name):
    if name not in _progs:
        _progs[name] = _build_k1() if name == "k1" else _build_k2()
    return _progs[name]


def _run_k1(query, emb, trace=False):
    nc = _get("k1")
    q = np.ascontiguousarray(query, dtype=np.float32)
    in_maps = [
        {"q": q, "e": np.ascontiguousarray(emb[c * CSH:(c + 1) * CSH])}
        for c in range(NCORES)
    ]
    return run_bass_kernel_spmd(
        nc, in_maps, core_ids=list(range(NCORES)), trace=trace
    )


def _run_k2(in_maps, trace=False):
    nc = _get("k2")
    return run_bass_kernel_spmd(
        nc, in_maps, core_ids=list(range(NCORES)), trace=trace
    )


def kernel(query, episode_embeddings, episodes, compressed_episodes,
           is_compressed, k, _trace=False, _results=None):
    assert int(k) == K
    r1 = _run_k1(query, episode_embeddings, trace=_trace)

    # host: merge the 8 per-shard candidate lists -> global top-8
    cand_s = np.concatenate(
        [r1.results[c]["scores"] for c in range(NCORES)], axis=1
    )  # [B, NCORES*NG1*K]
    goff = (np.arange(NG1 * K) // K) * 512   # group offset within a shard
    cand_i = np.concatenate(
        [r1.results[c]["idx"].astype(np.int64) + goff[None, :] + c * CSH
         for c in range(NCORES)],
        axis=1,
    )
    order = np.argsort(-cand_s, axis=1, kind="stable")[:, :K]
    top_scores = np.take_along_axis(cand_s, order, axis=1)
    top_idx = np.take_along_axis(cand_i, order, axis=1)  # [B, K] global

    # host: per-core gather/scatter index tensors
    comp = np.asarray(is_compressed).astype(bool)
    flat_idx = top_idx.reshape(-1)              # [B*K] winner slot w -> episode
    flat_comp = comp[flat_idx]
    owner = flat_idx // CSH
    ep_half = np.asarray(episodes, dtype=np.float32).reshape(C, 2, CROW)
    cp_flat = np.asarray(compressed_episodes, dtype=np.float32).reshape(C, CROW)

    in2 = []
    owned = []
    for c in range(NCORES):
        w = np.nonzero(owner == c)[0]           # winner slots owned by core c
        n = len(w)
        assert n <= CAP, f"core {c} owns {n} winners > capacity {CAP}"
        li = (flat_idx[w] - c * CSH).astype(np.int64)   # local table rows
        fc = flat_comp[w]
        # spread winners across batches, and within each batch interleave
        # the active rows evenly over all 128 partitions (SDMA engine balance)
        j = np.arange(n)
        b = j % NB
        i = j // NB
        mb = np.array([(n - bb + NB - 1) // NB for bb in range(NB)])
        mb = np.maximum(mb, 1)
        p = (i * 128) // mb[b]
        rank = b * 128 + p
        owned.append((w, rank))
        OOBT = np.int32(3 * CSH)
        OOBS = np.int32(2 * CAP)
        g0v = np.full(CAP, OOBT, np.int32)
        g1v = np.full(CAP, OOBT, np.int32)
        s0v = np.full(CAP, OOBS, np.int32)
        s1v = np.full(CAP, OOBS, np.int32)
        g0v[rank] = np.where(fc, 2 * CSH + li, 2 * li)
        g1v[rank] = np.where(fc, OOBT, 2 * li + 1)
        s0v[rank] = 2 * rank
        s1v[rank] = np.where(fc, OOBS, 2 * rank + 1)
        tbl = np.concatenate(
            [ep_half[c * CSH:(c + 1) * CSH].reshape(2 * CSH, CROW),
             cp_flat[c * CSH:(c + 1) * CSH]], axis=0
        )
        # device tensor idxs[p, t, bt]; rank = bt*128 + p
        iv = np.stack([g0v, g1v, s0v, s1v], axis=1).reshape(NB, 128, 4)
        in2.append({
            "tbl": tbl,
            "idxs": np.ascontiguousarray(iv.transpose(1, 2, 0)),
        })

    r2 = _run_k2(in2, trace=_trace)

    retrieved = np.empty((B * K, S, H), dtype=np.float32)
    for c in range(NCORES):
        w, rank = owned[c]
        retrieved[w] = r2.results[c]["out"][rank].reshape(-1, S, H)
    retrieved = retrieved.reshape(B, K, S, H)

    if _results is not None:
        _results["r1"] = r1
        _results["r2"] = r2
    return retrieved, top_scores.astype(np.float32)
